# revision 14
# baseline (speedup 1.0000x reference)
"""Trainium2 Bass kernel: per-sample position-decay mask multiply.

out[b, l, h] = data[b, l, h] * mask[b, l]
  mask[b, l] = 1 - (a_end - l)/C           if l < a_end
             = 1 - (l - a_idx)/C           elif l < sents_len
             = 0                           otherwise
  with a_end = aspect_Index + aspect_len, C = 40.

Sharding: data-parallel over the batch dim. Each of the 8 cores gets 64
samples, viewed as 128 partitions (2 half-sequences of 256 positions per
sample) x 25600 fp32. The mask is computed on-device ([128, 256]) from an
iota over positions plus 4 per-partition scalars, then broadcast-multiplied
into the data stream (tiles of [128, 3200]).
"""

import numpy as np

import concourse.bacc as bacc
import concourse.bass as bass
import concourse.mybir as mybir
import concourse.tile as tile
from concourse.bass_utils import run_bass_kernel_spmd

N_CORES = 8
B, L, H = 512, 512, 100
BS = B // N_CORES          # 64 samples per core
T = 2                      # half-sequences per sample
P = T * BS                 # 128 partitions
LT = L // T                # 256 positions per half-sequence
X = LT * H                 # 25600 fp32 per partition row
C = 40.0
W = 3200                   # main-loop tile width (32 positions x 100 feats)
LW = W // H                # positions per tile
NT = X // W                # number of tiles

F32 = mybir.dt.float32


def build_bass():
    nc = bacc.Bacc("TRN2", target_bir_lowering=False, debug=False)

    data = nc.dram_tensor("data", [BS, T, X], F32, kind="ExternalInput")
    out = nc.dram_tensor("out", [BS, T, X], F32, kind="ExternalOutput")
    # Per-partition scalars, pre-tiled on host to partition order p = t*BS + b.
    # Columns: 0: aec = a_end - C   (so (l - aec)/C == 1 - (a_end - l)/C)
    #          1: aic = a_idx + C   (so -(l - aic)/C == 1 - (l - a_idx)/C)
    #          2: aend = a_end, 3: slen = sents_len (comparison bounds)
    scals = nc.dram_tensor("scals", [P, 4], F32, kind="ExternalInput")

    # Partition order p = t*BS + b: DMA pairs the flattened [T, BS, W] DRAM
    # pattern with the [128, W] SBUF tile element-by-element.
    data_r = data.ap().transpose([1, 0, 2])
    out_r = out.ap().transpose([1, 0, 2])

    with tile.TileContext(nc) as tc:
        with (
            tc.tile_pool(name="consts", bufs=1) as consts,
            tc.tile_pool(name="io", bufs=4) as io,
        ):
            scal_t = consts.tile([P, 4], F32, tag="scals")
            nc.sync.dma_start(scal_t[:], scals.ap())

            # iota over positions: row p holds l = (p // BS)*LT + j
            iota_i = consts.tile([P, LT], mybir.dt.int32, tag="iota_i")
            nc.gpsimd.iota(
                iota_i[0:BS, :], pattern=[[1, LT]], base=0,
                channel_multiplier=0,
            )
            nc.gpsimd.iota(
                iota_i[BS:P, :], pattern=[[1, LT]], base=LT,
                channel_multiplier=0,
            )
            # Funnel: single DVE copy waits on gpsimd; everything after is
            # same-engine (DVE) ordered, so each op needs <=1 sem wait.
            iota_f = consts.tile([P, LT], F32, tag="iota_f")
            nc.vector.tensor_copy(iota_f[:], iota_i[:])

            # mask = where(l < aend, (l - aec)/C, where(l < slen, -(l - aic)/C, 0))
            mask_t = consts.tile([P, LT], F32, tag="mask")
            t1 = consts.tile([P, LT], F32, tag="t1")
            c2 = consts.tile([P, LT], F32, tag="c2")
            c1 = consts.tile([P, LT], mybir.dt.uint8, tag="c1")

            def col(k):
                return scal_t[:, k:k + 1].broadcast_to([P, LT])

            nc.vector.tensor_tensor(out=t1[:], in0=iota_f[:], in1=col(0),
                                    op=mybir.AluOpType.subtract)
            nc.vector.tensor_scalar(
                out=t1[:], in0=t1[:], scalar1=1.0 / C, scalar2=None,
                op0=mybir.AluOpType.mult,
            )
            nc.vector.tensor_tensor(out=mask_t[:], in0=iota_f[:], in1=col(1),
                                    op=mybir.AluOpType.subtract)
            nc.vector.tensor_scalar(
                out=mask_t[:], in0=mask_t[:], scalar1=-1.0 / C, scalar2=None,
                op0=mybir.AluOpType.mult,
            )
            nc.vector.tensor_tensor(out=c2[:], in0=iota_f[:], in1=col(3),
                                    op=mybir.AluOpType.is_lt)
            nc.vector.tensor_tensor(out=mask_t[:], in0=mask_t[:], in1=c2[:],
                                    op=mybir.AluOpType.mult)
            nc.vector.tensor_tensor(out=c1[:], in0=iota_f[:], in1=col(2),
                                    op=mybir.AluOpType.is_lt)
            nc.vector.copy_predicated(mask_t[:], c1[:], t1[:])

            for i in range(NT):
                t = io.tile([P, W], F32, tag="io")
                nc.sync.dma_start(t[:], data_r[:, :, i * W:(i + 1) * W])
                d3 = t[:].rearrange("p (l h) -> p l h", h=H)
                m3 = mask_t[:, i * LW:(i + 1) * LW].unsqueeze(2).broadcast_to(
                    [P, LW, H]
                )
                nc.vector.tensor_tensor(out=d3, in0=d3, in1=m3,
                                        op=mybir.AluOpType.mult)
                nc.sync.dma_start(out_r[:, :, i * W:(i + 1) * W], t[:])

    nc.compile()
    return nc


_NC = None


def _get_nc():
    global _NC
    if _NC is None:
        _NC = build_bass()
    return _NC


def make_in_maps(data, aspect_Index, aspect_len, sents_len):
    data = np.ascontiguousarray(np.asarray(data, dtype=np.float32))
    a_idx = np.asarray(aspect_Index).astype(np.float64)
    a_end = a_idx + np.asarray(aspect_len).astype(np.float64)
    s_len = np.asarray(sents_len).astype(np.float64)

    in_maps = []
    for c in range(N_CORES):
        sl = slice(c * BS, (c + 1) * BS)
        aend_v = np.tile(a_end[sl], T).astype(np.float32)[:, None]
        aidx_v = np.tile(a_idx[sl], T).astype(np.float32)[:, None]
        slen_v = np.tile(s_len[sl], T).astype(np.float32)[:, None]
        scal = np.concatenate(
            [aend_v - np.float32(C), aidx_v + np.float32(C), aend_v, slen_v],
            axis=1,
        ).astype(np.float32)
        in_maps.append({
            "data": data[sl].reshape(BS, T, X),
            "scals": np.ascontiguousarray(scal),
        })
    return in_maps


def kernel(data, aspect_Index, aspect_len, sents_len):
    nc = _get_nc()
    in_maps = make_in_maps(data, aspect_Index, aspect_len, sents_len)
    res = run_bass_kernel_spmd(nc, in_maps, list(range(N_CORES)))
    out = np.empty((B, L, H), dtype=np.float32)
    for c in range(N_CORES):
        out[c * BS:(c + 1) * BS] = res.results[c]["out"].reshape(BS, L, H)
    return out


# revision 19
# speedup vs baseline: 5.7342x; 5.7342x over previous
"""Trainium2 Bass kernel: per-sample position-decay mask multiply.

out[b, l, h] = data[b, l, h] * mask[b, l]
  mask[b, l] = 1 - (a_end - l)/C           if l < a_end
             = 1 - (l - a_idx)/C           elif l < sents_len
             = 0                           otherwise
  with a_end = aspect_Index + aspect_len, C = 40.

Sharding: data-parallel over the batch dim. Each of the 8 cores gets 64
samples, viewed as 128 partitions (2 half-sequences of 256 positions per
sample) x 25600 fp32. The mask is computed on-device ([128, 256]) from an
iota over positions plus 4 per-partition scalars, then broadcast-multiplied
into the data stream (tiles of [128, 3200]).
"""

import numpy as np

import concourse.bacc as bacc
import concourse.bass as bass
import concourse.mybir as mybir
import concourse.tile as tile
from concourse.bass_utils import run_bass_kernel_spmd

N_CORES = 8
B, L, H = 512, 512, 100
BS = B // N_CORES          # 64 samples per core
T = 2                      # half-sequences per sample
P = T * BS                 # 128 partitions
LT = L // T                # 256 positions per half-sequence
X = LT * H                 # 25600 fp32 per partition row
C = 40.0
W = 3200                   # main-loop tile width (32 positions x 100 feats)
LW = W // H                # positions per tile
NT = X // W                # number of tiles

F32 = mybir.dt.float32


def build_bass():
    nc = bacc.Bacc("TRN2", target_bir_lowering=False, debug=False)

    # Partition p = 2*b + t (natural contiguous view of the batch shard), so
    # the DMA access patterns are plain 2D [128, W] — the HWDGE splits the
    # outer (partition) dim across all 16 SDMA engines. The per-half position
    # offset (t*LT) is folded into the per-partition scalars on the host, so
    # on-device positions are just j = 0..LT-1 in every row.
    data = nc.dram_tensor("data", [P, X], F32, kind="ExternalInput")
    out = nc.dram_tensor("out", [P, X], F32, kind="ExternalOutput")
    # Columns (host precomputed, with off = (p % 2)*LT absorbed):
    #   0: aec = a_end - C - off   (so (j - aec)/C == 1 - (a_end - l)/C)
    #   1: aic = a_idx + C - off   (so -(j - aic)/C == 1 - (l - a_idx)/C)
    #   2: a_end - off, 3: sents_len - off (comparison bounds)
    scals = nc.dram_tensor("scals", [P, 4], F32, kind="ExternalInput")

    data_r = data.ap()
    out_r = out.ap()

    with tile.TileContext(nc) as tc:
        with (
            tc.tile_pool(name="consts", bufs=1) as consts,
            tc.tile_pool(name="io", bufs=4) as io,
        ):
            scal_t = consts.tile([P, 4], F32, tag="scals")
            nc.sync.dma_start(scal_t[:], scals.ap())

            # iota over local positions j = 0..LT-1, same in every row
            iota_i = consts.tile([P, LT], mybir.dt.int32, tag="iota_i")
            nc.gpsimd.iota(
                iota_i[:], pattern=[[1, LT]], base=0,
                channel_multiplier=0,
            )
            # Funnel: single DVE copy waits on gpsimd; everything after is
            # same-engine (DVE) ordered, so each op needs <=1 sem wait.
            iota_f = consts.tile([P, LT], F32, tag="iota_f")
            nc.vector.tensor_copy(iota_f[:], iota_i[:])

            # mask = where(l < aend, (l - aec)/C, where(l < slen, -(l - aic)/C, 0))
            mask_t = consts.tile([P, LT], F32, tag="mask")
            t1 = consts.tile([P, LT], F32, tag="t1")
            c2 = consts.tile([P, LT], F32, tag="c2")
            c1 = consts.tile([P, LT], mybir.dt.uint8, tag="c1")

            def col(k):
                return scal_t[:, k:k + 1].broadcast_to([P, LT])

            nc.vector.tensor_tensor(out=t1[:], in0=iota_f[:], in1=col(0),
                                    op=mybir.AluOpType.subtract)
            nc.vector.tensor_scalar(
                out=t1[:], in0=t1[:], scalar1=1.0 / C, scalar2=None,
                op0=mybir.AluOpType.mult,
            )
            nc.vector.tensor_tensor(out=mask_t[:], in0=iota_f[:], in1=col(1),
                                    op=mybir.AluOpType.subtract)
            nc.vector.tensor_scalar(
                out=mask_t[:], in0=mask_t[:], scalar1=-1.0 / C, scalar2=None,
                op0=mybir.AluOpType.mult,
            )
            nc.vector.tensor_tensor(out=c2[:], in0=iota_f[:], in1=col(3),
                                    op=mybir.AluOpType.is_lt)
            nc.vector.tensor_tensor(out=mask_t[:], in0=mask_t[:], in1=c2[:],
                                    op=mybir.AluOpType.mult)
            nc.vector.tensor_tensor(out=c1[:], in0=iota_f[:], in1=col(2),
                                    op=mybir.AluOpType.is_lt)
            nc.vector.copy_predicated(mask_t[:], c1[:], t1[:])

            for i in range(NT):
                t = io.tile([P, W], F32, tag="io")
                # loads on the SP HWDGE ring, stores on the ACT ring — the
                # two FIFOs issue concurrently
                nc.sync.dma_start(t[:], data_r[:, i * W:(i + 1) * W])
                d3 = t[:].rearrange("p (l h) -> p l h", h=H)
                m3 = mask_t[:, i * LW:(i + 1) * LW].unsqueeze(2).broadcast_to(
                    [P, LW, H]
                )
                nc.vector.tensor_tensor(out=d3, in0=d3, in1=m3,
                                        op=mybir.AluOpType.mult)
                nc.scalar.dma_start(out_r[:, i * W:(i + 1) * W], t[:])

    nc.compile()
    return nc


_NC = None


def _get_nc():
    global _NC
    if _NC is None:
        _NC = build_bass()
    return _NC


def make_in_maps(data, aspect_Index, aspect_len, sents_len):
    data = np.ascontiguousarray(np.asarray(data, dtype=np.float32))
    a_idx = np.asarray(aspect_Index).astype(np.float64)
    a_end = a_idx + np.asarray(aspect_len).astype(np.float64)
    s_len = np.asarray(sents_len).astype(np.float64)

    # partition p = 2*b_local + t; off[p] = (p % 2) * LT folds the
    # half-sequence position offset into the scalars (exact: small ints in f32)
    off = np.tile(np.array([0.0, float(LT)], dtype=np.float64), BS)  # [P]

    in_maps = []
    for c in range(N_CORES):
        sl = slice(c * BS, (c + 1) * BS)
        aend_v = np.repeat(a_end[sl], T) - off   # [P]
        aidx_v = np.repeat(a_idx[sl], T) - off
        slen_v = np.repeat(s_len[sl], T) - off
        scal = np.stack(
            [aend_v - C, aidx_v + C, aend_v, slen_v], axis=1,
        ).astype(np.float32)
        in_maps.append({
            "data": data[sl].reshape(P, X),
            "scals": np.ascontiguousarray(scal),
        })
    return in_maps


def kernel(data, aspect_Index, aspect_len, sents_len):
    nc = _get_nc()
    in_maps = make_in_maps(data, aspect_Index, aspect_len, sents_len)
    res = run_bass_kernel_spmd(nc, in_maps, list(range(N_CORES)))
    out = np.empty((B, L, H), dtype=np.float32)
    for c in range(N_CORES):
        out[c * BS:(c + 1) * BS] = res.results[c]["out"].reshape(BS, L, H)
    return out


if __name__ == "__main__":
    rng = np.random.default_rng(1)
    d = rng.standard_normal((B, L, H), dtype=np.float32)
    ai = rng.integers(0, 100, B).astype(np.int64)
    al = rng.integers(0, 10, B).astype(np.int64)
    slv = rng.integers(0, 512, B).astype(np.int64)
    got = kernel(d, ai, al, slv)
    i = np.arange(L, dtype=np.float32)[None, :]
    ae = (ai + al).astype(np.float32)[:, None]
    aif = ai.astype(np.float32)[:, None]
    m = np.where(i < ae, 1.0 - (ae - i) / C,
                 np.where(i < slv[:, None], 1.0 - (i - aif) / C, 0.0))
    want = d * m[:, :, None].astype(np.float32)
    print("selftest max abs err:", np.abs(got - want).max())
